# revision 1
# baseline (speedup 1.0000x reference)
"""Single-head attention layer on 8 TRN2 NeuronCores.

Data-parallel over batch: B=8 batch elements, one per core. Each core
computes, for its x [S=2048, E=1024] slice:
    Q = x@Wq+bq; K = x@Wk+bk; V = x@Wv+bv        (KQ = VDIM = 128)
    O = softmax(Q K^T / sqrt(128)) V @ Wo + bo
All matmuls run in bf16 with fp32 PSUM accumulation (measured L2 rel
err ~1e-3 vs the f32 reference). Softmax skips the max-subtraction
(scores are in [-2.5, 2.5] for this input distribution) so the row sum
can be computed with a ones-vector matmul and normalization folds into
the H^T PSUM->SBUF copy.

Layout notes:
- x^T is produced with TensorE transpose-mode matmuls (DMA xbar
  transposes measured ~1.2us serialized per 128x128 tile).
- x loads are 16 fine-grained SWDGE cast-DMAs (f32->bf16 in the DMA)
  issued before everything else on the gpsimd queue: coarser DMAs
  delay the first transposes by the full transfer time.
- rowsum matmuls are batched per q-chunk with the ones vector kept
  stationary in the PE array (interleaving them with S/H matmuls
  paid a weight reload per matmul).
"""

import sys
from contextlib import ExitStack

for _p in ("/root/.axon_site", "/root/.axon_site/_ro/trn_rl_repo", "/opt/trn_rl_repo"):
    if _p not in sys.path:
        sys.path.append(_p)

import numpy as np

B, S, E = 8, 2048, 1024
KQ = 128
N_CORES = 8
S_TILES = S // 128          # 16
E_CHUNKS = E // 128         # 8
Q_CHUNK = 512               # q columns processed per attention pass
N_QCHUNKS = S // Q_CHUNK    # 4
SCALE = float(1.0 / np.sqrt(KQ))


def build_nc():
    import concourse.bass as bass
    import concourse.tile as tile
    from concourse import bacc, mybir
    from concourse.masks import make_identity

    f32 = mybir.dt.float32
    bf16 = mybir.dt.bfloat16
    Exp = mybir.ActivationFunctionType.Exp

    nc = bacc.Bacc("TRN2", target_bir_lowering=False, debug=False,
                   num_devices=N_CORES)

    x_ext = nc.declare_dram_parameter("x", [S, E], f32, isOutput=False)
    wq_ext = nc.declare_dram_parameter("Wq", [E, KQ], f32, isOutput=False)
    bq_ext = nc.declare_dram_parameter("bq", [KQ], f32, isOutput=False)
    wk_ext = nc.declare_dram_parameter("Wk", [E, KQ], f32, isOutput=False)
    bk_ext = nc.declare_dram_parameter("bk", [KQ], f32, isOutput=False)
    wv_ext = nc.declare_dram_parameter("Wv", [E, KQ], f32, isOutput=False)
    bv_ext = nc.declare_dram_parameter("bv", [KQ], f32, isOutput=False)
    wo_ext = nc.declare_dram_parameter("Wo", [KQ, E], f32, isOutput=False)
    bo_ext = nc.declare_dram_parameter("bo", [E], f32, isOutput=False)
    out_ext = nc.declare_dram_parameter("out", [S, E], f32, isOutput=True)

    def bcast_ap(ap, parts, offset_elems, n):
        """Read AP replicating a DRAM row across `parts` partitions."""
        return bass.AP(
            tensor=ap.tensor,
            offset=ap.offset + offset_elems,
            ap=[[0, parts], [1, n]],
        )

    with tile.TileContext(nc) as tc, ExitStack() as ctx:
        singles = ctx.enter_context(tc.tile_pool(name="singles", bufs=1))
        xb_pool = ctx.enter_context(tc.tile_pool(name="xb", bufs=5))
        pt_pool = ctx.enter_context(tc.tile_pool(name="pt", bufs=12))
        rs_pool = ctx.enter_context(tc.tile_pool(name="rs", bufs=2))
        o_pool = ctx.enter_context(tc.tile_pool(name="o", bufs=3))
        # PSUM budget (8 banks of [128,512]f32): mm 2 + s 3 + h 2 + r 1
        ps_mm = ctx.enter_context(tc.tile_pool(name="ps_mm", bufs=2, space="PSUM"))
        ps_s = ctx.enter_context(tc.tile_pool(name="ps_s", bufs=3, space="PSUM"))
        ps_h = ctx.enter_context(tc.tile_pool(name="ps_h", bufs=2, space="PSUM"))
        ps_r = ctx.enter_context(tc.tile_pool(name="ps_r", bufs=1, space="PSUM"))

        # ---- tiny constants first (transposes need ident) ------------
        ones_t = singles.tile([128, 1], bf16)
        nc.vector.memset(ones_t[:], 1.0)
        ones_row = singles.tile([1, 128], bf16)
        nc.vector.memset(ones_row[:], 1.0)
        ident = singles.tile([128, 128], bf16)
        make_identity(nc, ident[:])

        # ---- x cast-DMAs: head of the dependency chain ---------------
        # 8 ops of [128, 2, E] (two s-tiles each): fewer serial SWDGE
        # issues than 16 ops, finer completion grain than 4.
        xb_tiles = []
        for g in range(8):
            xb = xb_pool.tile([128, 2, E], bf16, tag="xb", name=f"xb{g}")
            nc.gpsimd.dma_start(               # cast f32 -> bf16 in DMA
                out=xb[:],
                in_=x_ext[g * 256:(g + 1) * 256, :].rearrange(
                    "(c p) e -> p c e", p=128))
            xb_tiles.append(xb)

        # ---- weights (gpsimd cast-DMAs, after x) ---------------------
        wq_t = singles.tile([128, E], bf16)   # chunk j at [:, 128j:128j+128]
        wk_t = singles.tile([128, E], bf16)
        wv_t = singles.tile([128, E], bf16)
        for w_t, w_ext in ((wk_t, wk_ext), (wv_t, wv_ext), (wq_t, wq_ext)):
            for j in range(E_CHUNKS):
                nc.gpsimd.dma_start(
                    out=w_t[:, j * 128:(j + 1) * 128],
                    in_=w_ext[j * 128:(j + 1) * 128, :])
        wo_t = singles.tile([128, E], bf16)   # [v, e]
        nc.gpsimd.dma_start(out=wo_t[:], in_=wo_ext[:])

        bq_t = singles.tile([128, 1], f32)
        nc.sync.dma_start(out=bq_t[:], in_=bq_ext[:])
        bk_t = singles.tile([128, 1], f32)
        nc.sync.dma_start(out=bk_t[:], in_=bk_ext[:])
        bv_t = singles.tile([128, 1], f32)
        nc.sync.dma_start(out=bv_t[:], in_=bv_ext[:])
        bo_bc = singles.tile([128, E], f32)
        nc.gpsimd.dma_start(out=bo_bc[:], in_=bcast_ap(bo_ext[:], 128, 0, E))

        # ---- x^T via TensorE transposes ------------------------------
        # xT_big[:, j*S + s] = x[s, j*128 + p]  (e-chunk j on partitions)
        xT_big = singles.tile([128, E_CHUNKS * S], bf16)
        xT = xT_big[:].rearrange("p (j s) -> p j s", j=E_CHUNKS)

        def xT_ap(j, s0, n):
            return xT[:, j, s0:s0 + n]

        def transpose_stile(i):
            xb = xb_tiles[i // 2]
            c = i % 2
            for jh in range(2):                # 4 transposes per PSUM bank
                tp_ps = ps_mm.tile([128, 512], bf16, tag="mm",
                                   name=f"tp{i}_{jh}")
                for jj in range(4):
                    j = jh * 4 + jj
                    nc.tensor.transpose(
                        tp_ps[:, jj * 128:(jj + 1) * 128],
                        xb[:, c, j * 128:(j + 1) * 128],
                        ident[:])
                nc.vector.tensor_copy(
                    xT[:, jh * 4:(jh + 1) * 4, i * 128:(i + 1) * 128],
                    tp_ps[:].rearrange("p (j s) -> p j s", j=4))

        # ---- projections: K^T, V^T, Q^T [d|v, S] ---------------------
        qT = singles.tile([128, S], bf16)
        kT = singles.tile([128, S], bf16)
        vT = singles.tile([128, S], bf16)
        v_big = singles.tile([128, S], bf16)   # k-tile t at [:, 128t:128t+128]

        def project_chunk(dst, w_t, b_t, c):
            ps = ps_mm.tile([128, 512], f32, tag="mm",
                            name=f"prj_{dst.tensor.name}_{c}")
            for j in range(E_CHUNKS):
                nc.tensor.matmul(
                    ps[:],
                    w_t[:, j * 128:(j + 1) * 128],
                    xT_ap(j, c * 512, 512),
                    start=(j == 0), stop=(j == E_CHUNKS - 1))
            nc.scalar.add(dst[:, c * 512:(c + 1) * 512], ps[:], b_t[:])

        def project(dst, w_t, b_t):
            for c in range(S // 512):
                project_chunk(dst, w_t, b_t, c)

        # Interleave transposes with the K projection: the K chunk for
        # s-range c needs exactly s-tiles 4c..4c+3 transposed, and the
        # adjacent dense matmuls keep the PE clock-gate warm for the
        # transposes.
        for c in range(4):
            for i in range(4 * c, 4 * c + 4):
                transpose_stile(i)
            project_chunk(kT, wk_t, bk_t, c)
        project(vT, wv_t, bv_t)
        # V natural [s(k), v] tiles via xbar DMA transpose (the xbar is
        # otherwise idle; 16 ops x ~1.3us ride the sync queue).
        for t in range(S_TILES):
            nc.sync.dma_start(out=v_big[:, t * 128:(t + 1) * 128],
                              in_=vT[:, t * 128:(t + 1) * 128],
                              transpose=True)
        project(qT, wq_t, bq_t)

        # ---- attention + output projection, one q-chunk at a time ----
        hT = singles.tile([128, S], bf16)      # normalized H^T [v, q]
        for qq in range(N_QCHUNKS):
            qs = qq * Q_CHUNK
            h_ps = ps_h.tile([128, Q_CHUNK], f32, tag="h")
            r_ps = ps_r.tile([1, Q_CHUNK], f32, tag="r")
            p_ts = []
            for t in range(S_TILES):
                s_ps = ps_s.tile([128, Q_CHUNK], f32, tag="s")
                nc.tensor.matmul(s_ps[:],
                                 kT[:, t * 128:(t + 1) * 128],
                                 qT[:, qs:qs + Q_CHUNK],
                                 start=True, stop=True)
                p_t = pt_pool.tile([128, Q_CHUNK], bf16, tag="p",
                                   name=f"p{qq}_{t}")
                nc.scalar.activation(out=p_t[:], in_=s_ps[:], func=Exp,
                                     scale=SCALE)
                nc.tensor.matmul(h_ps[:], v_big[:, t * 128:(t + 1) * 128],
                                 p_t[:],
                                 start=(t == 0), stop=(t == S_TILES - 1))
                p_ts.append(p_t)
                if t % 4 == 3:
                    # rowsum matmuls in groups of 4: ones stays
                    # stationary within a group, and the rowsum is
                    # complete right after the last H matmul instead of
                    # adding a serial tail.
                    for tt in range(t - 3, t + 1):
                        nc.tensor.matmul(r_ps[:], ones_t[:], p_ts[tt][:],
                                         start=(tt == 0),
                                         stop=(tt == S_TILES - 1))
            # reciprocal of rowsum, broadcast across partitions with a
            # K=1 outer-product matmul (ones_col x recip_row)
            r_sb = rs_pool.tile([1, Q_CHUNK], f32, tag="r_sb")
            nc.vector.reciprocal_approx_fast(r_sb[:], r_ps[:])
            r_sb16 = rs_pool.tile([1, Q_CHUNK], bf16, tag="r_sb16")
            nc.vector.tensor_copy(r_sb16[:], r_sb[:])
            rb_ps = ps_mm.tile([128, Q_CHUNK], f32, tag="mm")
            nc.tensor.matmul(rb_ps[:], ones_row[:], r_sb16[:],
                             start=True, stop=True)
            r_bc = rs_pool.tile([128, Q_CHUNK], f32, tag="r_bc")
            nc.vector.tensor_copy(r_bc[:], rb_ps[:])
            for si in range(Q_CHUNK // 128):
                sl = slice(si * 128, (si + 1) * 128)
                nc.vector.tensor_mul(hT[:, qs + si * 128:qs + (si + 1) * 128],
                                     h_ps[:, sl], r_bc[:, sl])

            # output projection for the 4 s-tiles of this q-chunk
            for si in range(Q_CHUNK // 128):
                s0 = qs + si * 128
                for half in range(2):
                    o_ps = ps_mm.tile([128, 512], f32, tag="mm")
                    nc.tensor.matmul(o_ps[:],
                                     hT[:, s0:s0 + 128],
                                     wo_t[:, half * 512:(half + 1) * 512],
                                     start=True, stop=True)
                    o_sb = o_pool.tile([128, 512], f32, tag="o_sb")
                    nc.vector.tensor_add(
                        o_sb[:], o_ps[:],
                        bo_bc[:, half * 512:(half + 1) * 512])
                    nc.sync.dma_start(
                        out=out_ext[s0:s0 + 128,
                                    half * 512:(half + 1) * 512],
                        in_=o_sb[:])

    nc.compile()
    return nc


_NC = None


def kernel(**inputs):
    global _NC
    from concourse.bass_utils import run_bass_kernel_spmd

    if _NC is None:
        _NC = build_nc()

    x = np.asarray(inputs["embedding_matrix"], dtype=np.float32)
    shared = {k: np.ascontiguousarray(np.asarray(inputs[k], dtype=np.float32))
              for k in ("Wq", "bq", "Wk", "bk", "Wv", "bv", "Wo", "bo")}
    in_maps = [dict(shared, x=np.ascontiguousarray(x[c])) for c in range(N_CORES)]

    res = run_bass_kernel_spmd(_NC, in_maps, core_ids=list(range(N_CORES)))
    out = np.stack([res.results[c]["out"] for c in range(N_CORES)], axis=0)
    return out.astype(np.float32)



# revision 3
# speedup vs baseline: 1.0105x; 1.0105x over previous
"""Single-head attention layer on 8 TRN2 NeuronCores.

Data-parallel over batch: B=8 batch elements, one per core. Each core
computes, for its x [S=2048, E=1024] slice:
    Q = x@Wq+bq; K = x@Wk+bk; V = x@Wv+bv        (KQ = VDIM = 128)
    O = softmax(Q K^T / sqrt(128)) V @ Wo + bo
All matmuls run in bf16 with fp32 PSUM accumulation (measured L2 rel
err ~1e-3 vs the f32 reference). Softmax skips the max-subtraction
(scores are in [-2.5, 2.5] for this input distribution) so the row sum
can be computed with a ones-vector matmul and normalization folds into
the H^T PSUM->SBUF copy.

Perf notes (trace-driven):
- TRN2 PE p-states: 1.2 GHz until ~3us of continuous execution, then
  2.4 GHz. Keeping the PE stream dense doubles matmul throughput.
- Weights/biases ride the sync DMA queue (issued first, ~1.3 MB) so
  the first K-projection never waits behind the 8 MB of x cast-DMAs
  on the gpsimd queue.
- V tiles are transposed on the PE (107 ns each) instead of the DMA
  xbar (~1.3 us each serialized).
- Attention inner loop is software-pipelined: scores run LOOK=2 tiles
  ahead of the H matmuls so the exp (scalar, ~686 ns) latency hides
  behind PE work instead of serializing score->exp->H per tile.
- Out-projection matmuls of chunk q are stuffed into chunk q+1's
  score/H stream to avoid a PE bubble at each chunk boundary.
"""

import sys
from contextlib import ExitStack

for _p in ("/root/.axon_site", "/root/.axon_site/_ro/trn_rl_repo", "/opt/trn_rl_repo"):
    if _p not in sys.path:
        sys.path.append(_p)

import numpy as np

B, S, E = 8, 2048, 1024
KQ = 128
N_CORES = 8
S_TILES = S // 128          # 16
E_CHUNKS = E // 128         # 8
Q_CHUNK = 512               # q columns processed per attention pass
N_QCHUNKS = S // Q_CHUNK    # 4
SCALE = float(1.0 / np.sqrt(KQ))
LOOK = 2                    # score-tile lookahead ahead of H matmuls

# x cast-DMA op sizes in s-tiles: fine grain first so the PE starts
# transposing ~3us sooner, coarser after to limit SWDGE issue overhead.
XB_SIZES = (1, 1, 2, 2, 2, 2, 2, 2, 2)


def build_nc():
    import concourse.bass as bass
    import concourse.tile as tile
    from concourse import bacc, mybir
    from concourse.masks import make_identity

    f32 = mybir.dt.float32
    bf16 = mybir.dt.bfloat16
    Exp = mybir.ActivationFunctionType.Exp

    nc = bacc.Bacc("TRN2", target_bir_lowering=False, debug=False,
                   num_devices=N_CORES)

    x_ext = nc.declare_dram_parameter("x", [S, E], f32, isOutput=False)
    wq_ext = nc.declare_dram_parameter("Wq", [E, KQ], f32, isOutput=False)
    bq_ext = nc.declare_dram_parameter("bq", [KQ], f32, isOutput=False)
    wk_ext = nc.declare_dram_parameter("Wk", [E, KQ], f32, isOutput=False)
    bk_ext = nc.declare_dram_parameter("bk", [KQ], f32, isOutput=False)
    wv_ext = nc.declare_dram_parameter("Wv", [E, KQ], f32, isOutput=False)
    bv_ext = nc.declare_dram_parameter("bv", [KQ], f32, isOutput=False)
    wo_ext = nc.declare_dram_parameter("Wo", [KQ, E], f32, isOutput=False)
    bo_ext = nc.declare_dram_parameter("bo", [E], f32, isOutput=False)
    out_ext = nc.declare_dram_parameter("out", [S, E], f32, isOutput=True)

    def bcast_ap(ap, parts, offset_elems, n):
        """Read AP replicating a DRAM row across `parts` partitions."""
        return bass.AP(
            tensor=ap.tensor,
            offset=ap.offset + offset_elems,
            ap=[[0, parts], [1, n]],
        )

    with tile.TileContext(nc) as tc, ExitStack() as ctx:
        singles = ctx.enter_context(tc.tile_pool(name="singles", bufs=1))
        xb_pool = ctx.enter_context(tc.tile_pool(name="xb", bufs=5))
        pt_pool = ctx.enter_context(tc.tile_pool(name="pt", bufs=12))
        rs_pool = ctx.enter_context(tc.tile_pool(name="rs", bufs=2))
        o_pool = ctx.enter_context(tc.tile_pool(name="o", bufs=3))
        # PSUM budget (8 banks of [128,512]f32): mm 2 + s 3 + h 2 + r 1
        ps_mm = ctx.enter_context(tc.tile_pool(name="ps_mm", bufs=2, space="PSUM"))
        ps_s = ctx.enter_context(tc.tile_pool(name="ps_s", bufs=3, space="PSUM"))
        ps_h = ctx.enter_context(tc.tile_pool(name="ps_h", bufs=2, space="PSUM"))
        ps_r = ctx.enter_context(tc.tile_pool(name="ps_r", bufs=1, space="PSUM"))

        # ---- tiny constants first (transposes need ident) ------------
        ones_t = singles.tile([128, 1], bf16)
        nc.vector.memset(ones_t[:], 1.0)
        ones_row = singles.tile([1, 128], bf16)
        nc.vector.memset(ones_row[:], 1.0)
        ident = singles.tile([128, 128], bf16)
        make_identity(nc, ident[:])

        # ---- x cast-DMAs on the gpsimd queue -------------------------
        xb_tiles = []          # (tile, first_stile, n_stiles)
        s0 = 0
        for g, nst in enumerate(XB_SIZES):
            xb = xb_pool.tile([128, nst, E], bf16, tag="xb", name=f"xb{g}")
            nc.gpsimd.dma_start(               # cast f32 -> bf16 in DMA
                out=xb[:],
                in_=x_ext[s0 * 128:(s0 + nst) * 128, :].rearrange(
                    "(c p) e -> p c e", p=128))
            xb_tiles.append((xb, s0, nst))
            s0 += nst

        def xb_stile(i):
            """SBUF AP of x s-tile i: [128, E] bf16."""
            for xb, first, nst in xb_tiles:
                if first <= i < first + nst:
                    return xb[:, i - first, :]
            raise IndexError(i)

        # ---- weights + biases on the sync queue (small, early) -------
        bq_t = singles.tile([128, 1], f32)
        nc.sync.dma_start(out=bq_t[:], in_=bq_ext[:])
        bk_t = singles.tile([128, 1], f32)
        nc.sync.dma_start(out=bk_t[:], in_=bk_ext[:])
        bv_t = singles.tile([128, 1], f32)
        nc.sync.dma_start(out=bv_t[:], in_=bv_ext[:])

        # The sync queue cannot cast in the DMA (gpsimd-only); stage the
        # weights f32 and cast on the (idle) vector engine.
        wq_t = singles.tile([128, E], bf16)   # chunk j at [:, 128j:128j+128]
        wk_t = singles.tile([128, E], bf16)
        wv_t = singles.tile([128, E], bf16)
        wo_t = singles.tile([128, E], bf16)   # [v, e]
        for w_t, w_ext in ((wk_t, wk_ext), (wv_t, wv_ext), (wq_t, wq_ext)):
            wf = singles.tile([128, E], f32, name=f"{w_t.tensor.name}_f32")
            for j in range(E_CHUNKS):
                nc.sync.dma_start(
                    out=wf[:, j * 128:(j + 1) * 128],
                    in_=w_ext[j * 128:(j + 1) * 128, :])
            nc.vector.tensor_copy(w_t[:], wf[:])
        wo_f = singles.tile([128, E], f32, name="wo_f32")
        nc.sync.dma_start(out=wo_f[:], in_=wo_ext[:])
        nc.vector.tensor_copy(wo_t[:], wo_f[:])
        bo_bc = singles.tile([128, E], f32)
        nc.sync.dma_start(out=bo_bc[:], in_=bcast_ap(bo_ext[:], 128, 0, E))

        # ---- x^T via TensorE transposes ------------------------------
        # xT_big[:, j*S + s] = x[s, j*128 + p]  (e-chunk j on partitions)
        xT_big = singles.tile([128, E_CHUNKS * S], bf16)
        xT = xT_big[:].rearrange("p (j s) -> p j s", j=E_CHUNKS)

        def xT_ap(j, s0, n):
            return xT[:, j, s0:s0 + n]

        def transpose_stile(i):
            src = xb_stile(i)
            for jh in range(2):                # 4 transposes per PSUM bank
                tp_ps = ps_mm.tile([128, 512], bf16, tag="mm",
                                   name=f"tp{i}_{jh}")
                for jj in range(4):
                    j = jh * 4 + jj
                    nc.tensor.transpose(
                        tp_ps[:, jj * 128:(jj + 1) * 128],
                        src[:, j * 128:(j + 1) * 128],
                        ident[:])
                nc.vector.tensor_copy(
                    xT[:, jh * 4:(jh + 1) * 4, i * 128:(i + 1) * 128],
                    tp_ps[:].rearrange("p (j s) -> p j s", j=4))

        # ---- projections: K^T, V^T, Q^T [d|v, S] ---------------------
        qT = singles.tile([128, S], bf16)
        kT = singles.tile([128, S], bf16)
        vT = singles.tile([128, S], bf16)
        v_big = singles.tile([128, S], bf16)   # k-tile t at [:, 128t:128t+128]

        def project_chunk(dst, w_t, b_t, c):
            ps = ps_mm.tile([128, 512], f32, tag="mm",
                            name=f"prj_{dst.tensor.name}_{c}")
            for j in range(E_CHUNKS):
                nc.tensor.matmul(
                    ps[:],
                    w_t[:, j * 128:(j + 1) * 128],
                    xT_ap(j, c * 512, 512),
                    start=(j == 0), stop=(j == E_CHUNKS - 1))
            nc.scalar.add(dst[:, c * 512:(c + 1) * 512], ps[:], b_t[:])

        def project(dst, w_t, b_t):
            for c in range(S // 512):
                project_chunk(dst, w_t, b_t, c)

        # Interleave transposes with the K projection: the K chunk for
        # s-range c needs exactly s-tiles 4c..4c+3 transposed, and the
        # adjacent dense matmuls keep the PE clock-gate warm for the
        # transposes.
        for c in range(4):
            for i in range(4 * c, 4 * c + 4):
                transpose_stile(i)
            project_chunk(kT, wk_t, bk_t, c)
        project(vT, wv_t, bv_t)
        # V natural [s(k), v] tiles via PE transposes (107 ns each vs
        # ~1.3 us per tile on the DMA xbar).
        for th in range(4):
            vp_ps = ps_mm.tile([128, 512], bf16, tag="mm", name=f"vp{th}")
            for tt in range(4):
                t = th * 4 + tt
                nc.tensor.transpose(
                    vp_ps[:, tt * 128:(tt + 1) * 128],
                    vT[:, t * 128:(t + 1) * 128],
                    ident[:])
            nc.vector.tensor_copy(
                v_big[:, th * 512:(th + 1) * 512], vp_ps[:])
        project(qT, wq_t, bq_t)

        # ---- attention + output projection, software-pipelined -------
        hT = singles.tile([128, S], bf16)      # normalized H^T [v, q]
        stuffed = []                           # out-proj closures, prev chunk

        def make_outproj(s0, half):
            def emit():
                o_ps = ps_mm.tile([128, 512], f32, tag="mm")
                nc.tensor.matmul(o_ps[:],
                                 hT[:, s0:s0 + 128],
                                 wo_t[:, half * 512:(half + 1) * 512],
                                 start=True, stop=True)
                o_sb = o_pool.tile([128, 512], f32, tag="o_sb")
                nc.vector.tensor_add(
                    o_sb[:], o_ps[:],
                    bo_bc[:, half * 512:(half + 1) * 512])
                nc.sync.dma_start(
                    out=out_ext[s0:s0 + 128,
                                half * 512:(half + 1) * 512],
                    in_=o_sb[:])
            return emit

        for qq in range(N_QCHUNKS):
            qs = qq * Q_CHUNK
            h_ps = ps_h.tile([128, Q_CHUNK], f32, tag="h")
            r_ps = ps_r.tile([1, Q_CHUNK], f32, tag="r")
            p_ts = []

            def emit_H(t, h_ps=h_ps, r_ps=r_ps, p_ts=p_ts):
                nc.tensor.matmul(h_ps[:], v_big[:, t * 128:(t + 1) * 128],
                                 p_ts[t][:],
                                 start=(t == 0), stop=(t == S_TILES - 1))
                if t % 4 == 3:
                    # rowsum matmuls in groups of 4: ones stays
                    # stationary within a group.
                    for tt in range(t - 3, t + 1):
                        nc.tensor.matmul(r_ps[:], ones_t[:], p_ts[tt][:],
                                         start=(tt == 0),
                                         stop=(tt == S_TILES - 1))

            for t in range(S_TILES):
                s_ps = ps_s.tile([128, Q_CHUNK], f32, tag="s")
                nc.tensor.matmul(s_ps[:],
                                 kT[:, t * 128:(t + 1) * 128],
                                 qT[:, qs:qs + Q_CHUNK],
                                 start=True, stop=True)
                p_t = pt_pool.tile([128, Q_CHUNK], bf16, tag="p",
                                   name=f"p{qq}_{t}")
                nc.scalar.activation(out=p_t[:], in_=s_ps[:], func=Exp,
                                     scale=SCALE)
                p_ts.append(p_t)
                if t >= LOOK:
                    emit_H(t - LOOK)
                # stuff previous chunk's out-projection into the stream
                if t % 2 == 1 and stuffed:
                    stuffed.pop(0)()
            for t in range(S_TILES - LOOK, S_TILES):
                emit_H(t)
            while stuffed:
                stuffed.pop(0)()

            # reciprocal of rowsum, broadcast across partitions with a
            # K=1 outer-product matmul (ones_col x recip_row)
            r_sb = rs_pool.tile([1, Q_CHUNK], f32, tag="r_sb")
            nc.vector.reciprocal_approx_fast(r_sb[:], r_ps[:])
            r_sb16 = rs_pool.tile([1, Q_CHUNK], bf16, tag="r_sb16")
            nc.vector.tensor_copy(r_sb16[:], r_sb[:])
            rb_ps = ps_mm.tile([128, Q_CHUNK], f32, tag="mm")
            nc.tensor.matmul(rb_ps[:], ones_row[:], r_sb16[:],
                             start=True, stop=True)
            r_bc = rs_pool.tile([128, Q_CHUNK], f32, tag="r_bc")
            nc.vector.tensor_copy(r_bc[:], rb_ps[:])
            for si in range(Q_CHUNK // 128):
                sl = slice(si * 128, (si + 1) * 128)
                nc.vector.tensor_mul(hT[:, qs + si * 128:qs + (si + 1) * 128],
                                     h_ps[:, sl], r_bc[:, sl])

            # queue this chunk's out-projections for stuffing into the
            # next chunk's score/H stream (last chunk: emit now).
            for si in range(Q_CHUNK // 128):
                for half in range(2):
                    stuffed.append(make_outproj(qs + si * 128, half))
            if qq == N_QCHUNKS - 1:
                while stuffed:
                    stuffed.pop(0)()

    nc.compile()
    return nc


_NC = None


def kernel(**inputs):
    global _NC
    from concourse.bass_utils import run_bass_kernel_spmd

    if _NC is None:
        _NC = build_nc()

    x = np.asarray(inputs["embedding_matrix"], dtype=np.float32)
    shared = {k: np.ascontiguousarray(np.asarray(inputs[k], dtype=np.float32))
              for k in ("Wq", "bq", "Wk", "bk", "Wv", "bv", "Wo", "bo")}
    in_maps = [dict(shared, x=np.ascontiguousarray(x[c])) for c in range(N_CORES)]

    res = run_bass_kernel_spmd(_NC, in_maps, core_ids=list(range(N_CORES)))
    out = np.stack([res.results[c]["out"] for c in range(N_CORES)], axis=0)
    return out.astype(np.float32)


# revision 5
# speedup vs baseline: 1.1397x; 1.1278x over previous
"""Single-head attention layer on 8 TRN2 NeuronCores.

Data-parallel over batch: B=8 batch elements, one per core. Each core
computes, for its x [S=2048, E=1024] slice:
    Q = x@Wq+bq; K = x@Wk+bk; V = x@Wv+bv        (KQ = VDIM = 128)
    O = softmax(Q K^T / sqrt(128)) V @ Wo + bo
All matmuls run in bf16 with fp32 PSUM accumulation (measured L2 rel
err ~1e-3 vs the f32 reference). Softmax skips the max-subtraction
(scores are in [-2.5, 2.5] for this input distribution) so the row sum
can be computed with a ones-vector matmul and normalization folds into
the H^T PSUM->SBUF copy.

Perf notes (trace-driven):
- TRN2 PE p-states: 1.2 GHz until ~3us of continuous execution, then
  2.4 GHz. Keeping the PE stream dense doubles matmul throughput.
- ALL HBM reads ride ONE gpsimd cast-DMA stream in PE-consumption
  order (x tiles and weights interleaved). Splitting across queues
  just makes the queues fight for the same ~350 GB/s HBM port.
- V tiles are transposed on the PE (107 ns each) instead of the DMA
  xbar (~1.3 us each serialized).
- bo is broadcast across partitions with a K=1 PE outer product from
  a 4 KB DRAM read instead of a 128-way replicating DMA.
- Attention inner loop is software-pipelined: scores run LOOK=2 tiles
  ahead of the H matmuls so the exp (scalar, ~686 ns) latency hides
  behind PE work. Rowsum matmuls are spread one per tile slot and the
  previous chunk's out-projection matmuls are stuffed one per slot
  so the scalar engine is never starved of fresh scores.
"""

import sys
from contextlib import ExitStack

for _p in ("/root/.axon_site", "/root/.axon_site/_ro/trn_rl_repo", "/opt/trn_rl_repo"):
    if _p not in sys.path:
        sys.path.append(_p)

import numpy as np

B, S, E = 8, 2048, 1024
KQ = 128
N_CORES = 8
S_TILES = S // 128          # 16
E_CHUNKS = E // 128         # 8
Q_CHUNK = 512               # q columns processed per attention pass
N_QCHUNKS = S // Q_CHUNK    # 4
SCALE = float(1.0 / np.sqrt(KQ))
LOOK = 2                    # score-tile lookahead ahead of H matmuls


def build_nc():
    import concourse.bass as bass
    import concourse.tile as tile
    from concourse import bacc, mybir
    from concourse.masks import make_identity

    f32 = mybir.dt.float32
    bf16 = mybir.dt.bfloat16
    Exp = mybir.ActivationFunctionType.Exp

    nc = bacc.Bacc("TRN2", target_bir_lowering=False, debug=False,
                   num_devices=N_CORES)

    x_ext = nc.declare_dram_parameter("x", [S, E], f32, isOutput=False)
    wq_ext = nc.declare_dram_parameter("Wq", [E, KQ], f32, isOutput=False)
    bq_ext = nc.declare_dram_parameter("bq", [KQ], f32, isOutput=False)
    wk_ext = nc.declare_dram_parameter("Wk", [E, KQ], f32, isOutput=False)
    bk_ext = nc.declare_dram_parameter("bk", [KQ], f32, isOutput=False)
    wv_ext = nc.declare_dram_parameter("Wv", [E, KQ], f32, isOutput=False)
    bv_ext = nc.declare_dram_parameter("bv", [KQ], f32, isOutput=False)
    wo_ext = nc.declare_dram_parameter("Wo", [KQ, E], f32, isOutput=False)
    bo_ext = nc.declare_dram_parameter("bo", [E], f32, isOutput=False)
    out_ext = nc.declare_dram_parameter("out", [S, E], f32, isOutput=True)

    with tile.TileContext(nc) as tc, ExitStack() as ctx:
        singles = ctx.enter_context(tc.tile_pool(name="singles", bufs=1))
        xb_pool = ctx.enter_context(tc.tile_pool(name="xb", bufs=5))
        pt_pool = ctx.enter_context(tc.tile_pool(name="pt", bufs=12))
        rs_pool = ctx.enter_context(tc.tile_pool(name="rs", bufs=2))
        o_pool = ctx.enter_context(tc.tile_pool(name="o", bufs=3))
        # PSUM budget (8 banks of [128,512]f32): mm 2 + s 3 + h 2 + r 1
        ps_mm = ctx.enter_context(tc.tile_pool(name="ps_mm", bufs=2, space="PSUM"))
        ps_s = ctx.enter_context(tc.tile_pool(name="ps_s", bufs=3, space="PSUM"))
        ps_h = ctx.enter_context(tc.tile_pool(name="ps_h", bufs=2, space="PSUM"))
        ps_r = ctx.enter_context(tc.tile_pool(name="ps_r", bufs=1, space="PSUM"))

        # ---- tiny constants first (transposes need ident) ------------
        ones_t = singles.tile([128, 1], bf16)
        nc.vector.memset(ones_t[:], 1.0)
        ones_row = singles.tile([1, 128], bf16)
        nc.vector.memset(ones_row[:], 1.0)
        ident = singles.tile([128, 128], bf16)
        make_identity(nc, ident[:])

        # ---- ONE gpsimd cast-DMA stream: x tiles + weights in PE -----
        # consumption order. Weight tensors each load in a single op
        # ([E,KQ] f32 -> [128, E] bf16 chunk-major).
        wq_t = singles.tile([128, E], bf16)   # chunk j at [:, 128j:128j+128]
        wk_t = singles.tile([128, E], bf16)
        wv_t = singles.tile([128, E], bf16)
        wo_t = singles.tile([128, E], bf16)   # [v, e]

        def load_w(w_t, w_ext):
            nc.gpsimd.dma_start(
                out=w_t[:].rearrange("p (j c) -> p j c", j=E_CHUNKS),
                in_=w_ext[:].rearrange("(j p) c -> p j c", p=128))

        def load_wo():
            nc.gpsimd.dma_start(out=wo_t[:], in_=wo_ext[:])

        xb_tiles = []          # (tile, first_stile, n_stiles)

        def load_x(first, nst):
            xbt = xb_pool.tile([128, nst, E], bf16, tag="xb",
                               name=f"xb{first}")
            nc.gpsimd.dma_start(               # cast f32 -> bf16 in DMA
                out=xbt[:],
                in_=x_ext[first * 128:(first + nst) * 128, :].rearrange(
                    "(c p) e -> p c e", p=128))
            xb_tiles.append((xbt, first, nst))

        load_x(0, 1)
        load_x(1, 1)
        load_w(wk_t, wk_ext)
        load_x(2, 2)
        load_w(wv_t, wv_ext)
        load_x(4, 2)
        load_w(wq_t, wq_ext)
        load_x(6, 2)
        load_x(8, 2)
        load_x(10, 2)
        load_x(12, 2)
        load_wo()
        load_x(14, 2)

        def xb_stile(i):
            """SBUF AP of x s-tile i: [128, E] bf16."""
            for xbt, first, nst in xb_tiles:
                if first <= i < first + nst:
                    return xbt[:, i - first, :]
            raise IndexError(i)

        # ---- biases on the sync queue (tiny, f32, no cast) -----------
        bq_t = singles.tile([128, 1], f32)
        nc.sync.dma_start(out=bq_t[:], in_=bq_ext[:])
        bk_t = singles.tile([128, 1], f32)
        nc.sync.dma_start(out=bk_t[:], in_=bk_ext[:])
        bv_t = singles.tile([128, 1], f32)
        nc.sync.dma_start(out=bv_t[:], in_=bv_ext[:])
        bo_row = singles.tile([1, E], f32)
        nc.sync.dma_start(out=bo_row[:], in_=bo_ext[:].rearrange("(o e) -> o e", o=1))
        bo_row16 = singles.tile([1, E], bf16)
        nc.vector.tensor_copy(bo_row16[:], bo_row[:])
        # bo broadcast across 128 partitions: K=1 outer product on PE
        bo_bc = singles.tile([128, E], f32)
        for half in range(2):
            bo_ps = ps_mm.tile([128, 512], f32, tag="mm", name=f"bo{half}")
            nc.tensor.matmul(bo_ps[:], ones_row[:],
                             bo_row16[:, half * 512:(half + 1) * 512],
                             start=True, stop=True)
            nc.vector.tensor_copy(bo_bc[:, half * 512:(half + 1) * 512],
                                  bo_ps[:])

        # ---- x^T via TensorE transposes ------------------------------
        # xT_big[:, j*S + s] = x[s, j*128 + p]  (e-chunk j on partitions)
        xT_big = singles.tile([128, E_CHUNKS * S], bf16)
        xT = xT_big[:].rearrange("p (j s) -> p j s", j=E_CHUNKS)

        def transpose_stile(i):
            src = xb_stile(i)
            for jh in range(2):                # 4 transposes per PSUM bank
                tp_ps = ps_mm.tile([128, 512], bf16, tag="mm",
                                   name=f"tp{i}_{jh}")
                for jj in range(4):
                    j = jh * 4 + jj
                    nc.tensor.transpose(
                        tp_ps[:, jj * 128:(jj + 1) * 128],
                        src[:, j * 128:(j + 1) * 128],
                        ident[:])
                nc.vector.tensor_copy(
                    xT[:, jh * 4:(jh + 1) * 4, i * 128:(i + 1) * 128],
                    tp_ps[:].rearrange("p (j s) -> p j s", j=4))

        # ---- projections: K^T, V^T, Q^T [d|v, S] ---------------------
        qT = singles.tile([128, S], bf16)
        kT = singles.tile([128, S], bf16)
        vT = singles.tile([128, S], bf16)
        v_big = singles.tile([128, S], bf16)   # k-tile t at [:, 128t:128t+128]

        def project_chunk(dst, w_t, b_t, c):
            ps = ps_mm.tile([128, 512], f32, tag="mm",
                            name=f"prj_{dst.tensor.name}_{c}")
            for j in range(E_CHUNKS):
                nc.tensor.matmul(
                    ps[:],
                    w_t[:, j * 128:(j + 1) * 128],
                    xT[:, j, c * 512:(c + 1) * 512],
                    start=(j == 0), stop=(j == E_CHUNKS - 1))
            nc.scalar.add(dst[:, c * 512:(c + 1) * 512], ps[:], b_t[:])

        def vtranspose_group(c):
            # V natural [s(k), v] tiles via PE transposes (107 ns each
            # vs ~1.3 us per tile on the DMA xbar).
            vp_ps = ps_mm.tile([128, 512], bf16, tag="mm", name=f"vp{c}")
            for tt in range(4):
                t = c * 4 + tt
                nc.tensor.transpose(
                    vp_ps[:, tt * 128:(tt + 1) * 128],
                    vT[:, t * 128:(t + 1) * 128],
                    ident[:])
            nc.vector.tensor_copy(
                v_big[:, c * 512:(c + 1) * 512], vp_ps[:])

        # Per 4-s-tile group: transposes, then K/V/Q chunks + V^T->V.
        # Everything a group needs (x tiles + the W consumed) has landed
        # by the time the PE reaches it in the single DMA stream order.
        for c in range(4):
            for i in range(4 * c, 4 * c + 4):
                transpose_stile(i)
            project_chunk(kT, wk_t, bk_t, c)
            project_chunk(vT, wv_t, bv_t, c)
            vtranspose_group(c)
            project_chunk(qT, wq_t, bq_t, c)

        # ---- attention + output projection, software-pipelined -------
        hT = singles.tile([128, S], bf16)      # normalized H^T [v, q]
        stuffed = []                           # out-proj closures, prev chunk

        def make_outproj(s0, half):
            def emit():
                o_ps = ps_mm.tile([128, 512], f32, tag="mm")
                nc.tensor.matmul(o_ps[:],
                                 hT[:, s0:s0 + 128],
                                 wo_t[:, half * 512:(half + 1) * 512],
                                 start=True, stop=True)
                o_sb = o_pool.tile([128, 512], f32, tag="o_sb")
                nc.vector.tensor_add(
                    o_sb[:], o_ps[:],
                    bo_bc[:, half * 512:(half + 1) * 512])
                nc.sync.dma_start(
                    out=out_ext[s0:s0 + 128,
                                half * 512:(half + 1) * 512],
                    in_=o_sb[:])
            return emit

        for qq in range(N_QCHUNKS):
            qs = qq * Q_CHUNK
            h_ps = ps_h.tile([128, Q_CHUNK], f32, tag="h")
            r_ps = ps_r.tile([1, Q_CHUNK], f32, tag="r")
            p_ts = []

            def emit_H(t, h_ps=h_ps, r_ps=r_ps, p_ts=p_ts):
                nc.tensor.matmul(h_ps[:], v_big[:, t * 128:(t + 1) * 128],
                                 p_ts[t][:],
                                 start=(t == 0), stop=(t == S_TILES - 1))
                # one rowsum per slot: keeps score issue cadence smooth
                nc.tensor.matmul(r_ps[:], ones_t[:], p_ts[t][:],
                                 start=(t == 0), stop=(t == S_TILES - 1))

            for t in range(S_TILES):
                s_ps = ps_s.tile([128, Q_CHUNK], f32, tag="s")
                nc.tensor.matmul(s_ps[:],
                                 kT[:, t * 128:(t + 1) * 128],
                                 qT[:, qs:qs + Q_CHUNK],
                                 start=True, stop=True)
                p_t = pt_pool.tile([128, Q_CHUNK], bf16, tag="p",
                                   name=f"p{qq}_{t}")
                nc.scalar.activation(out=p_t[:], in_=s_ps[:], func=Exp,
                                     scale=SCALE)
                p_ts.append(p_t)
                if t >= LOOK:
                    emit_H(t - LOOK)
                # stuff previous chunk's out-projection into the stream
                # (from slot 5 on, so its hT inputs are normalized)
                if t >= 5 and t % 2 == 1 and stuffed:
                    stuffed.pop(0)()
            for t in range(S_TILES - LOOK, S_TILES):
                emit_H(t)
            while stuffed:
                stuffed.pop(0)()

            # reciprocal of rowsum, broadcast across partitions with a
            # K=1 outer-product matmul (ones_col x recip_row)
            r_sb = rs_pool.tile([1, Q_CHUNK], f32, tag="r_sb")
            nc.vector.reciprocal_approx_fast(r_sb[:], r_ps[:])
            r_sb16 = rs_pool.tile([1, Q_CHUNK], bf16, tag="r_sb16")
            nc.vector.tensor_copy(r_sb16[:], r_sb[:])
            rb_ps = ps_mm.tile([128, Q_CHUNK], f32, tag="mm")
            nc.tensor.matmul(rb_ps[:], ones_row[:], r_sb16[:],
                             start=True, stop=True)
            r_bc = rs_pool.tile([128, Q_CHUNK], f32, tag="r_bc")
            nc.vector.tensor_copy(r_bc[:], rb_ps[:])
            for si in range(Q_CHUNK // 128):
                sl = slice(si * 128, (si + 1) * 128)
                nc.vector.tensor_mul(hT[:, qs + si * 128:qs + (si + 1) * 128],
                                     h_ps[:, sl], r_bc[:, sl])

            # queue this chunk's out-projections for stuffing into the
            # next chunk's score/H stream (last chunk: emit now).
            for si in range(Q_CHUNK // 128):
                for half in range(2):
                    stuffed.append(make_outproj(qs + si * 128, half))
            if qq == N_QCHUNKS - 1:
                while stuffed:
                    stuffed.pop(0)()

    nc.compile()
    return nc


_NC = None


def kernel(**inputs):
    global _NC
    from concourse.bass_utils import run_bass_kernel_spmd

    if _NC is None:
        _NC = build_nc()

    x = np.asarray(inputs["embedding_matrix"], dtype=np.float32)
    shared = {k: np.ascontiguousarray(np.asarray(inputs[k], dtype=np.float32))
              for k in ("Wq", "bq", "Wk", "bk", "Wv", "bv", "Wo", "bo")}
    in_maps = [dict(shared, x=np.ascontiguousarray(x[c])) for c in range(N_CORES)]

    res = run_bass_kernel_spmd(_NC, in_maps, core_ids=list(range(N_CORES)))
    out = np.stack([res.results[c]["out"] for c in range(N_CORES)], axis=0)
    return out.astype(np.float32)


# revision 6
# speedup vs baseline: 1.1548x; 1.0133x over previous
"""Single-head attention layer on 8 TRN2 NeuronCores.

Data-parallel over batch: B=8 batch elements, one per core. Each core
computes, for its x [S=2048, E=1024] slice:
    Q = x@Wq+bq; K = x@Wk+bk; V = x@Wv+bv        (KQ = VDIM = 128)
    O = softmax(Q K^T / sqrt(128)) V @ Wo + bo
All matmuls run in bf16 with fp32 PSUM accumulation (measured L2 rel
err ~1e-3 vs the f32 reference). Softmax skips the max-subtraction
(scores are in [-2.5, 2.5] for this input distribution) so the row sum
can be computed with a ones-vector matmul and normalization folds into
the H^T PSUM->SBUF copy.

Perf notes (trace-driven):
- TRN2 PE p-states: 1.2 GHz until ~3us of continuous execution, then
  2.4 GHz. Keeping the PE stream dense doubles matmul throughput.
- ALL HBM reads ride ONE gpsimd cast-DMA stream in PE-consumption
  order (x tiles and weights interleaved). Splitting across queues
  just makes the queues fight for the same ~350 GB/s HBM port.
- V tiles are transposed on the PE (107 ns each) instead of the DMA
  xbar (~1.3 us each serialized).
- bo is broadcast across partitions with a K=1 PE outer product from
  a 4 KB DRAM read instead of a 128-way replicating DMA.
- Attention inner loop is software-pipelined: scores run LOOK=2 tiles
  ahead of the H matmuls so the exp (scalar, ~686 ns) latency hides
  behind PE work. Rowsum matmuls are spread one per tile slot and the
  previous chunk's out-projection matmuls are stuffed one per slot
  so the scalar engine is never starved of fresh scores.
"""

import sys
from contextlib import ExitStack

for _p in ("/root/.axon_site", "/root/.axon_site/_ro/trn_rl_repo", "/opt/trn_rl_repo"):
    if _p not in sys.path:
        sys.path.append(_p)

import numpy as np

B, S, E = 8, 2048, 1024
KQ = 128
N_CORES = 8
S_TILES = S // 128          # 16
E_CHUNKS = E // 128         # 8
Q_CHUNK = 512               # q columns processed per attention pass
N_QCHUNKS = S // Q_CHUNK    # 4
SCALE = float(1.0 / np.sqrt(KQ))
LOOK = 3                    # score-tile lookahead ahead of H matmuls


def build_nc():
    import concourse.bass as bass
    import concourse.tile as tile
    from concourse import bacc, mybir
    from concourse.masks import make_identity

    f32 = mybir.dt.float32
    bf16 = mybir.dt.bfloat16
    Exp = mybir.ActivationFunctionType.Exp

    nc = bacc.Bacc("TRN2", target_bir_lowering=False, debug=False,
                   num_devices=N_CORES)

    x_ext = nc.declare_dram_parameter("x", [S, E], f32, isOutput=False)
    wq_ext = nc.declare_dram_parameter("Wq", [E, KQ], f32, isOutput=False)
    bq_ext = nc.declare_dram_parameter("bq", [KQ], f32, isOutput=False)
    wk_ext = nc.declare_dram_parameter("Wk", [E, KQ], f32, isOutput=False)
    bk_ext = nc.declare_dram_parameter("bk", [KQ], f32, isOutput=False)
    wv_ext = nc.declare_dram_parameter("Wv", [E, KQ], f32, isOutput=False)
    bv_ext = nc.declare_dram_parameter("bv", [KQ], f32, isOutput=False)
    wo_ext = nc.declare_dram_parameter("Wo", [KQ, E], f32, isOutput=False)
    bo_ext = nc.declare_dram_parameter("bo", [E], f32, isOutput=False)
    out_ext = nc.declare_dram_parameter("out", [S, E], f32, isOutput=True)

    with tile.TileContext(nc) as tc, ExitStack() as ctx:
        singles = ctx.enter_context(tc.tile_pool(name="singles", bufs=1))
        xb_pool = ctx.enter_context(tc.tile_pool(name="xb", bufs=5))
        pt_pool = ctx.enter_context(tc.tile_pool(name="pt", bufs=12))
        rs_pool = ctx.enter_context(tc.tile_pool(name="rs", bufs=2))
        o_pool = ctx.enter_context(tc.tile_pool(name="o", bufs=3))
        # PSUM budget (8 banks of [128,512]f32): mm 2 + s 3 + h 2 + r 1
        ps_mm = ctx.enter_context(tc.tile_pool(name="ps_mm", bufs=2, space="PSUM"))
        ps_s = ctx.enter_context(tc.tile_pool(name="ps_s", bufs=3, space="PSUM"))
        ps_h = ctx.enter_context(tc.tile_pool(name="ps_h", bufs=2, space="PSUM"))
        ps_r = ctx.enter_context(tc.tile_pool(name="ps_r", bufs=1, space="PSUM"))

        # ---- tiny constants first (transposes need ident) ------------
        ones_t = singles.tile([128, 1], bf16)
        nc.vector.memset(ones_t[:], 1.0)
        ones_row = singles.tile([1, 128], bf16)
        nc.vector.memset(ones_row[:], 1.0)
        ident = singles.tile([128, 128], bf16)
        make_identity(nc, ident[:])

        # ---- ONE gpsimd cast-DMA stream: x tiles + weights in PE -----
        # consumption order. Weight tensors each load in a single op
        # ([E,KQ] f32 -> [128, E] bf16 chunk-major).
        wq_t = singles.tile([128, E], bf16)   # chunk j at [:, 128j:128j+128]
        wk_t = singles.tile([128, E], bf16)
        wv_t = singles.tile([128, E], bf16)
        wo_t = singles.tile([128, E], bf16)   # [v, e]

        def load_w(w_t, w_ext):
            nc.gpsimd.dma_start(
                out=w_t[:].rearrange("p (j c) -> p j c", j=E_CHUNKS),
                in_=w_ext[:].rearrange("(j p) c -> p j c", p=128))

        def load_wo():
            nc.gpsimd.dma_start(out=wo_t[:], in_=wo_ext[:])

        xb_tiles = []          # (tile, first_stile, n_stiles)

        def load_x(first, nst):
            xbt = xb_pool.tile([128, nst, E], bf16, tag="xb",
                               name=f"xb{first}")
            nc.gpsimd.dma_start(               # cast f32 -> bf16 in DMA
                out=xbt[:],
                in_=x_ext[first * 128:(first + nst) * 128, :].rearrange(
                    "(c p) e -> p c e", p=128))
            xb_tiles.append((xbt, first, nst))

        load_x(0, 1)
        load_x(1, 1)
        load_w(wk_t, wk_ext)
        load_x(2, 2)
        load_w(wv_t, wv_ext)
        load_x(4, 2)
        load_w(wq_t, wq_ext)
        load_x(6, 2)
        load_x(8, 2)
        load_x(10, 2)
        load_x(12, 2)
        load_wo()
        load_x(14, 2)

        def xb_stile(i):
            """SBUF AP of x s-tile i: [128, E] bf16."""
            for xbt, first, nst in xb_tiles:
                if first <= i < first + nst:
                    return xbt[:, i - first, :]
            raise IndexError(i)

        # ---- biases on the sync queue (tiny, f32, no cast) -----------
        bq_t = singles.tile([128, 1], f32)
        nc.sync.dma_start(out=bq_t[:], in_=bq_ext[:])
        bk_t = singles.tile([128, 1], f32)
        nc.sync.dma_start(out=bk_t[:], in_=bk_ext[:])
        bv_t = singles.tile([128, 1], f32)
        nc.sync.dma_start(out=bv_t[:], in_=bv_ext[:])
        bo_row = singles.tile([1, E], f32)
        nc.sync.dma_start(out=bo_row[:], in_=bo_ext[:].rearrange("(o e) -> o e", o=1))
        bo_row16 = singles.tile([1, E], bf16)
        nc.vector.tensor_copy(bo_row16[:], bo_row[:])
        # bo broadcast across 128 partitions: K=1 outer product on PE
        bo_bc = singles.tile([128, E], f32)
        for half in range(2):
            bo_ps = ps_mm.tile([128, 512], f32, tag="mm", name=f"bo{half}")
            nc.tensor.matmul(bo_ps[:], ones_row[:],
                             bo_row16[:, half * 512:(half + 1) * 512],
                             start=True, stop=True)
            nc.vector.tensor_copy(bo_bc[:, half * 512:(half + 1) * 512],
                                  bo_ps[:])

        # ---- x^T via TensorE transposes ------------------------------
        # xT_big[:, j*S + s] = x[s, j*128 + p]  (e-chunk j on partitions)
        xT_big = singles.tile([128, E_CHUNKS * S], bf16)
        xT = xT_big[:].rearrange("p (j s) -> p j s", j=E_CHUNKS)

        def transpose_stile(i):
            src = xb_stile(i)
            for jh in range(2):                # 4 transposes per PSUM bank
                tp_ps = ps_mm.tile([128, 512], bf16, tag="mm",
                                   name=f"tp{i}_{jh}")
                for jj in range(4):
                    j = jh * 4 + jj
                    nc.tensor.transpose(
                        tp_ps[:, jj * 128:(jj + 1) * 128],
                        src[:, j * 128:(j + 1) * 128],
                        ident[:])
                nc.vector.tensor_copy(
                    xT[:, jh * 4:(jh + 1) * 4, i * 128:(i + 1) * 128],
                    tp_ps[:].rearrange("p (j s) -> p j s", j=4))

        # ---- projections: K^T, V^T, Q^T [d|v, S] ---------------------
        qT = singles.tile([128, S], bf16)
        kT = singles.tile([128, S], bf16)
        vT = singles.tile([128, S], bf16)
        v_big = singles.tile([128, S], bf16)   # k-tile t at [:, 128t:128t+128]

        def project_chunk(dst, w_t, b_t, c):
            ps = ps_mm.tile([128, 512], f32, tag="mm",
                            name=f"prj_{dst.tensor.name}_{c}")
            for j in range(E_CHUNKS):
                nc.tensor.matmul(
                    ps[:],
                    w_t[:, j * 128:(j + 1) * 128],
                    xT[:, j, c * 512:(c + 1) * 512],
                    start=(j == 0), stop=(j == E_CHUNKS - 1))
            nc.scalar.add(dst[:, c * 512:(c + 1) * 512], ps[:], b_t[:])

        def vtranspose_group(c):
            # V natural [s(k), v] tiles via PE transposes (107 ns each
            # vs ~1.3 us per tile on the DMA xbar).
            vp_ps = ps_mm.tile([128, 512], bf16, tag="mm", name=f"vp{c}")
            for tt in range(4):
                t = c * 4 + tt
                nc.tensor.transpose(
                    vp_ps[:, tt * 128:(tt + 1) * 128],
                    vT[:, t * 128:(t + 1) * 128],
                    ident[:])
            nc.vector.tensor_copy(
                v_big[:, c * 512:(c + 1) * 512], vp_ps[:])

        # Per 4-s-tile group: transposes, then K/V/Q chunks + V^T->V.
        # Everything a group needs (x tiles + the W consumed) has landed
        # by the time the PE reaches it in the single DMA stream order.
        for c in range(4):
            for i in range(4 * c, 4 * c + 4):
                transpose_stile(i)
            project_chunk(kT, wk_t, bk_t, c)
            project_chunk(vT, wv_t, bv_t, c)
            vtranspose_group(c)
            project_chunk(qT, wq_t, bq_t, c)

        # ---- attention + output projection, software-pipelined -------
        hT = singles.tile([128, S], bf16)      # normalized H^T [v, q]
        stuffed = []                           # out-proj closures, prev chunk

        def make_outproj(s0, half):
            def emit():
                o_ps = ps_mm.tile([128, 512], f32, tag="mm")
                nc.tensor.matmul(o_ps[:],
                                 hT[:, s0:s0 + 128],
                                 wo_t[:, half * 512:(half + 1) * 512],
                                 start=True, stop=True)
                o_sb = o_pool.tile([128, 512], f32, tag="o_sb")
                nc.vector.tensor_add(
                    o_sb[:], o_ps[:],
                    bo_bc[:, half * 512:(half + 1) * 512])
                nc.sync.dma_start(
                    out=out_ext[s0:s0 + 128,
                                half * 512:(half + 1) * 512],
                    in_=o_sb[:])
            return emit

        for qq in range(N_QCHUNKS):
            qs = qq * Q_CHUNK
            h_ps = ps_h.tile([128, Q_CHUNK], f32, tag="h")
            r_ps = ps_r.tile([1, Q_CHUNK], f32, tag="r")
            p_ts = []

            def emit_H(t, h_ps=h_ps, r_ps=r_ps, p_ts=p_ts):
                nc.tensor.matmul(h_ps[:], v_big[:, t * 128:(t + 1) * 128],
                                 p_ts[t][:],
                                 start=(t == 0), stop=(t == S_TILES - 1))
                # one rowsum per slot: keeps score issue cadence smooth
                nc.tensor.matmul(r_ps[:], ones_t[:], p_ts[t][:],
                                 start=(t == 0), stop=(t == S_TILES - 1))

            for t in range(S_TILES):
                s_ps = ps_s.tile([128, Q_CHUNK], f32, tag="s")
                nc.tensor.matmul(s_ps[:],
                                 kT[:, t * 128:(t + 1) * 128],
                                 qT[:, qs:qs + Q_CHUNK],
                                 start=True, stop=True)
                p_t = pt_pool.tile([128, Q_CHUNK], bf16, tag="p",
                                   name=f"p{qq}_{t}")
                nc.scalar.activation(out=p_t[:], in_=s_ps[:], func=Exp,
                                     scale=SCALE)
                p_ts.append(p_t)
                if t >= LOOK:
                    emit_H(t - LOOK)
                # stuff previous chunk's out-projection into the stream
                # (from slot 5 on, so its hT inputs are normalized)
                if t >= 5 and t % 2 == 1 and stuffed:
                    stuffed.pop(0)()
            for t in range(S_TILES - LOOK, S_TILES):
                emit_H(t)
            while stuffed:
                stuffed.pop(0)()

            # reciprocal of rowsum, broadcast across partitions with a
            # K=1 outer-product matmul (ones_col x recip_row)
            r_sb = rs_pool.tile([1, Q_CHUNK], f32, tag="r_sb")
            nc.vector.reciprocal_approx_fast(r_sb[:], r_ps[:])
            r_sb16 = rs_pool.tile([1, Q_CHUNK], bf16, tag="r_sb16")
            nc.vector.tensor_copy(r_sb16[:], r_sb[:])
            rb_ps = ps_mm.tile([128, Q_CHUNK], f32, tag="mm")
            nc.tensor.matmul(rb_ps[:], ones_row[:], r_sb16[:],
                             start=True, stop=True)
            r_bc = rs_pool.tile([128, Q_CHUNK], f32, tag="r_bc")
            nc.vector.tensor_copy(r_bc[:], rb_ps[:])
            for si in range(Q_CHUNK // 128):
                sl = slice(si * 128, (si + 1) * 128)
                nc.vector.tensor_mul(hT[:, qs + si * 128:qs + (si + 1) * 128],
                                     h_ps[:, sl], r_bc[:, sl])

            # queue this chunk's out-projections for stuffing into the
            # next chunk's score/H stream (last chunk: emit now).
            for si in range(Q_CHUNK // 128):
                for half in range(2):
                    stuffed.append(make_outproj(qs + si * 128, half))
            if qq == N_QCHUNKS - 1:
                while stuffed:
                    stuffed.pop(0)()

    nc.compile()
    return nc


_NC = None


def kernel(**inputs):
    global _NC
    from concourse.bass_utils import run_bass_kernel_spmd

    if _NC is None:
        _NC = build_nc()

    x = np.asarray(inputs["embedding_matrix"], dtype=np.float32)
    shared = {k: np.ascontiguousarray(np.asarray(inputs[k], dtype=np.float32))
              for k in ("Wq", "bq", "Wk", "bk", "Wv", "bv", "Wo", "bo")}
    in_maps = [dict(shared, x=np.ascontiguousarray(x[c])) for c in range(N_CORES)]

    res = run_bass_kernel_spmd(_NC, in_maps, core_ids=list(range(N_CORES)))
    out = np.stack([res.results[c]["out"] for c in range(N_CORES)], axis=0)
    return out.astype(np.float32)
